# revision 7
# baseline (speedup 1.0000x reference)
"""Trainium2 Bass kernel for nn_Attention_org_single_85074712199391.

Channel-attention module. Reference math (per batch b, head h):
    Qc = emb1[b].reshape(N, 4, dq)[:, h]          # [N, 128]
    Kc = emb_all[b].reshape(N, 4, dk)[:, h]       # [N, 240]
    Q = Qc @ Wq[h].T ; K = Kc @ Wk.T ; V = Kc @ Wv.T
    scores = Q.T @ K / sqrt(KV)                   # [128, 240]
    probs = softmax(instnorm(scores), axis=-1)
    context = probs @ V.T                         # [128, N]
    O1 = permute/concat(context) @ Wo.T           # [N, 512]

Algebraic rewrite used here (exact):
    S_h      = Qc.T @ Kc                          # big contraction over N
    scores_h = (Wq[h]/sqrt(KV)) @ S_h @ Wk.T
    probs_h  = softmax over dk of rstd*scores_h   # mean cancels in softmax
    P2_h     = probs_h @ Wv                       # [128, 240]
    ctx_h    = P2_h @ Kc.T                        # [128, N]
    O1       = sum_h ctx_h.T @ Wo[:, h::4].T     # accumulate over heads

Each core owns one batch (weights replicated; no collectives). Per rep:
    A: stream e1/ea as bf16 (DMA-cast loads); accumulate S_h in PSUM and
       PE-transpose ea into a resident bf16 eaT for phase C.
    B: tiny fp32 scores path, instance-norm stats via ones-matmul,
       batched softmax across heads, P2.T in bf16.
    C: context matmuls off eaT, then the output projection accumulating
       over heads; store fp32.

The build is software-pipelined ACROSS reps: phase A of rep r+1 (DMA
bound, PE light) is interleaved in emission order with phase C of rep r
(PE bound, DMA light), with eaT/p2t/rec double-buffered by rep parity.
Per-engine instruction streams execute in emission order, so the weave
keeps both the DMA engines and the PE continuously busy in steady state.
"""

import sys

import numpy as np

try:
    import concourse.bass as bass
except ImportError:  # harness environments without the repo on sys.path
    sys.path.insert(0, "/opt/trn_rl_repo")
    import concourse.bass as bass

import concourse.bacc as bacc

import ml_dtypes
import concourse.mybir as mybir
import concourse.tile as tile
from concourse.bass_utils import run_bass_kernel_spmd

F32 = mybir.dt.float32
BF16 = mybir.dt.bfloat16
AF = mybir.ActivationFunctionType
ALU = mybir.AluOpType

B, N, C, KV, H = 8, 4096, 512, 960, 4
DQ, DK = C // 4, KV // 4          # 128, 240
PT = 128                          # partition tile
NT = N // PT                      # 32 row tiles
NG = 8                            # groups of 4 row tiles (512 rows each)
DCH = 8                           # KV split into 8 chunks of 120 partitions
CHW = KV // DCH                   # 120
KCH = 2                           # dk split for 240-deep contractions
KHW = DK // KCH                   # 120
EPS = 1e-5
NORM_CNT = float(DQ * DK)         # instance-norm element count

import os as _os
PA_BUFS = int(_os.environ.get("PA_BUFS", "3"))
TP_BUFS = int(_os.environ.get("TP_BUFS", "2"))
PS_BUFS = int(_os.environ.get("PS_BUFS", "4"))
CTX_BUFS = int(_os.environ.get("CTX_BUFS", "2"))
OSB_BUFS = int(_os.environ.get("OSB_BUFS", "2"))


class Pools:
    """All tile pools, created once; tiles cycle via tags across reps."""

    def __init__(self, tc, stk):
        self.tc = tc
        self.pW = stk.enter_context(tc.tile_pool(name="pW", bufs=1))
        self.pR = stk.enter_context(tc.tile_pool(name="pR", bufs=1))
        self.pA = stk.enter_context(tc.tile_pool(name="pA", bufs=PA_BUFS))
        self.pBs = stk.enter_context(tc.tile_pool(name="pBs", bufs=2))
        self.pC = stk.enter_context(tc.tile_pool(name="pC", bufs=1))
        # PSUM: s_ps 2 banks + tp 2 banks + shared ring 4 banks = 8
        self.psS = stk.enter_context(
            tc.tile_pool(name="psS", bufs=1, space="PSUM"))
        self.psT = stk.enter_context(
            tc.tile_pool(name="psT", bufs=TP_BUFS, space="PSUM"))
        self.psR = stk.enter_context(
            tc.tile_pool(name="psR", bufs=PS_BUFS, space="PSUM"))

    def ps(self, shape, dtype=F32, name="ps"):
        # shared ring tag: slot size is the max over allocations (one
        # 2 KiB bank, set by the [128, 512] f32 cx/o tiles)
        return self.psR.tile(shape, dtype, tag="ps", name=name)


class RepState:
    """Per-rep persistent tiles (tags cycle buffers across reps)."""

    def __init__(self, P):
        pR = P.pR
        # double-buffered by rep parity: written in A/B of rep r, read in
        # C of rep r which overlaps A of rep r+1
        self.eaT = pR.tile([CHW, DCH, N], BF16, tag="eaT", name="eaT", bufs=2)
        self.p2t = pR.tile([KHW, H, 2, DQ], BF16, tag="p2t", name="p2t",
                           bufs=2)
        self.rec = pR.tile([DQ, H], F32, tag="rec", name="rec", bufs=2)
        # lifetime contained within A(r)..B(r): single buffer is fine
        self.s_sb = pR.tile([DQ, H, DK], F32, tag="s_sb", name="s_sb")
        self.sc_all = pR.tile([DQ, H, DK], F32, tag="sc_all", name="sc_all")
        self.e_all = pR.tile([DQ, H, DK], BF16, tag="e_all", name="e_all")
        self.stats = pR.tile([DQ, H, 2], F32, tag="stats", name="stats")
        # PSUM S accumulators, one tile: head h at 1KiB offset so each
        # [*, h, :240] output region stays inside a single 2KiB bank
        self.s_ps = P.psS.tile([DQ, H, 256], F32, tag="s_ps", name="s_ps")

    def small(self, P, name):
        return P.pR.tile([DQ, H], F32, tag=name, name=name)


def build_nc(reps=1):
    nc = bacc.Bacc("TRN2", target_bir_lowering=False, debug=False)

    e1 = nc.dram_tensor("e1", [N, C], F32, kind="ExternalInput").ap()
    ea = nc.dram_tensor("ea", [N, KV], F32, kind="ExternalInput").ap()
    wqt = nc.dram_tensor("wqt", [DQ, H, DQ], F32, kind="ExternalInput").ap()
    wkt = nc.dram_tensor("wkt", [DK, DK], F32, kind="ExternalInput").ap()
    wvb = nc.dram_tensor("wvb", [DK, DK], BF16, kind="ExternalInput").ap()
    wotb = nc.dram_tensor("wotb", [DQ, H, C], BF16, kind="ExternalInput").ap()
    idb = nc.dram_tensor("idb", [PT, PT], BF16, kind="ExternalInput").ap()
    o1 = nc.dram_tensor("o1", [N, C], F32, kind="ExternalOutput").ap()
    aps = dict(e1=e1, ea=ea, wqt=wqt, wkt=wkt, wvb=wvb, wotb=wotb, idb=idb,
               o1=o1)

    from contextlib import ExitStack

    with tile.TileContext(nc) as tc, ExitStack() as stk:
        P = Pools(tc, stk)
        W = load_consts(tc, P, aps)

        prev = None
        for _ in range(reps):
            cur = RepState(P)
            emit_macro(tc, P, W, aps, cur, prev)
            prev = cur
        emit_tail(tc, P, W, aps, prev)

    nc.compile()
    return nc


def load_consts(tc, P, aps):
    nc = tc.nc
    pW = P.pW
    W = {}
    W["wqt"] = pW.tile([DQ, H, DQ], F32, tag="wqt_sb", name="wqt_sb")
    nc.sync.dma_start(W["wqt"][:], aps["wqt"][:])
    W["wkt"] = pW.tile([KHW, KCH, DK], F32, tag="wkt_sb", name="wkt_sb")
    W["wvb"] = pW.tile([KHW, KCH, DK], BF16, tag="wvb_sb", name="wvb_sb")
    for j in range(KCH):
        nc.sync.dma_start(W["wkt"][:, j, :],
                          aps["wkt"][j * KHW:(j + 1) * KHW, :])
        nc.sync.dma_start(W["wvb"][:, j, :],
                          aps["wvb"][j * KHW:(j + 1) * KHW, :])
    W["wotb"] = pW.tile([DQ, H, C], BF16, tag="wotb_sb", name="wotb_sb")
    nc.sync.dma_start(W["wotb"][:], aps["wotb"][:])
    W["idb"] = pW.tile([PT, PT], BF16, tag="idb_sb", name="idb_sb")
    nc.sync.dma_start(W["idb"][:], aps["idb"][:])
    W["ones"] = pW.tile([PT, PT], F32, tag="ones_sb", name="ones_sb")
    nc.vector.memset(W["ones"][:], 1.0)
    W["eps"] = pW.tile([PT, 1], F32, tag="eps_sb", name="eps_sb")
    nc.vector.memset(W["eps"][:], EPS)
    return W


def emit_a_group(tc, P, W, aps, st, ii):
    """Phase A group ii: DMA-cast 512 rows of e1/ea, S matmuls, eaT
    transposes with batched PSUM->SBUF evacuation alternating DVE/ACT."""
    nc = tc.nc
    prows = slice(ii * 4 * PT, (ii + 1) * 4 * PT)
    e1b = P.pA.tile([PT, 4, C], BF16, tag="e1b", name="e1b")
    nc.gpsimd.dma_start(
        e1b[:], aps["e1"][prows, :].rearrange("(a p) k -> p a k", p=PT))
    eab = P.pA.tile([PT, 4, KV], BF16, tag="eab", name="eab")
    nc.gpsimd.dma_start(
        eab[:], aps["ea"][prows, :].rearrange("(a p) k -> p a k", p=PT))
    for a in range(4):
        i = 4 * ii + a
        rows = slice(i * PT, (i + 1) * PT)
        for h in range(H):
            # s_ps packs heads {0,1} in PSUM bank 0 and {2,3} in bank 1.
            # A start=True matmul clears has_written for its WHOLE bank
            # (2 KiB zero region), so the clear may only be issued by the
            # first head in each bank; the second head's first matmul
            # overwrites where the bit is unset, which the bank-wide
            # clear just guaranteed. Symmetrically, stop (which ends the
            # bank's accumulation group) fires only on the last head.
            nc.tensor.matmul(
                st.s_ps[:, h, 0:DK],
                e1b[:, a, h * DQ:(h + 1) * DQ],
                eab[:, a, h * DK:(h + 1) * DK],
                start=(i == 0 and h % 2 == 0),
                stop=(i == NT - 1 and h % 2 == 1),
            )
        tp = P.psT.tile([CHW, DCH, PT], BF16, tag="tp", name="tp")
        for j in range(DCH):
            nc.tensor.transpose(tp[:, j, :], eab[:, a, j * CHW:(j + 1) * CHW],
                                W["idb"][:])
        # one batched 1024-element copy per row tile, alternating engines
        if i % 2 == 0:
            nc.vector.tensor_copy(st.eaT[:, :, rows], tp[:])
        else:
            nc.scalar.copy(st.eaT[:, :, rows], tp[:])


def emit_s_evac(tc, P, st):
    """Evacuate the S accumulators (one strided ACT copy). Emitted at the
    end of the macro that accumulated them, so the next rep's first S
    matmul (which reuses the banks) never waits on a not-yet-issued ACT
    instruction."""
    tc.nc.scalar.copy(st.s_sb[:], st.s_ps[:, :, 0:DK])


def emit_b_scores(tc, P, W, st, heads):
    """Phase B part: scores + instance-norm partial sums for `heads`."""
    nc = tc.nc
    for h in heads:
        # U.T = (S.T-chunks) @ (Wq_h.T/sqrt(KV))  [240k, 128e], fp32
        ut_sb = P.pBs.tile([KHW, KCH, DQ], F32, tag="ut_sb", name="ut_sb")
        for j in range(KCH):
            ut_ps = P.ps([KHW, DQ], name="ut_ps")
            nc.tensor.matmul(ut_ps[:], st.s_sb[:, h, j * KHW:(j + 1) * KHW],
                             W["wqt"][:, h, :], start=True, stop=True)
            nc.vector.tensor_copy(ut_sb[:, j, :], ut_ps[:])
        # scores = U @ Wk.T  [128e, 240ek], fp32
        sc_ps = P.ps([DQ, DK], name="sc_ps")
        for j in range(KCH):
            nc.tensor.matmul(sc_ps[:], ut_sb[:, j, :], W["wkt"][:, j, :],
                             start=(j == 0), stop=(j == KCH - 1))
        # evacuate + per-row sums of x and x^2 for instance-norm
        nc.scalar.activation(st.sc_all[:, h, :], sc_ps[:], AF.Copy,
                             accum_out=st.stats[:, h, 0:1])
        junk = P.pBs.tile([DQ, DK], F32, tag="junk", name="junk")
        nc.scalar.activation(junk[:], sc_ps[:], AF.Square,
                             accum_out=st.stats[:, h, 1:2])


def emit_b_softmax(tc, P, W, st):
    """Phase B part: instance-norm stats reduce + batched exp."""
    nc = tc.nc
    mu_all = st.small(P, "mu_all")
    m2_all = st.small(P, "m2_all")
    mu2_all = st.small(P, "mu2_all")
    var_all = st.small(P, "var_all")
    sd_all = st.small(P, "sd_all")
    rstd_all = st.small(P, "rstd_all")
    den_all = st.small(P, "den_all")

    # cross-partition reduce of stats; every partition gets totals
    tot_ps = P.ps([DQ, H, 2], name="tot_ps")
    nc.tensor.matmul(tot_ps[:], W["ones"][:], st.stats[:],
                     start=True, stop=True)
    nc.scalar.mul(mu_all[:], tot_ps[:, :, 0:1], 1.0 / NORM_CNT)
    nc.scalar.mul(m2_all[:], tot_ps[:, :, 1:2], 1.0 / NORM_CNT)
    nc.scalar.square(mu2_all[:], mu_all[:])
    nc.vector.tensor_sub(var_all[:], m2_all[:], mu2_all[:])
    nc.scalar.activation(sd_all[:], var_all[:], AF.Sqrt,
                         bias=W["eps"][:, 0:1])
    nc.vector.reciprocal(rstd_all[:], sd_all[:])
    # softmax over ek of rstd*scores: the mean shift cancels in softmax,
    # and no max-shift is needed -- scores are z-scored by rstd so
    # |exponent| stays ~<=8, far from fp32 overflow. The 1/denominator is
    # applied later as the ctx-evac scale.
    for h in range(H):
        nc.scalar.activation(st.e_all[:, h, :], st.sc_all[:, h, :],
                             AF.Exp, scale=rstd_all[:, h:h + 1],
                             accum_out=den_all[:, h:h + 1])
    nc.vector.reciprocal(st.rec[:], den_all[:])


def emit_b_p2(tc, P, W, st):
    """Phase B part: P2.T = (exp @ Wv).T in chunks (unnormalized)."""
    nc = tc.nc
    for h in range(H):
        pt_sb = P.pBs.tile([KHW, KCH, DQ], BF16, tag="pt_sb", name="pt_sb")
        for j in range(KCH):
            pt_ps = P.ps([KHW, DQ], BF16, name="pt_ps")
            nc.tensor.transpose(pt_ps[:], st.e_all[:, h, j * KHW:(j + 1) * KHW],
                                W["idb"][:])
            nc.vector.tensor_copy(pt_sb[:, j, :], pt_ps[:])
        for jd in range(2):
            p2t_ps = P.ps([CHW, DQ], name="p2t_ps")
            for jk in range(KCH):
                nc.tensor.matmul(
                    p2t_ps[:],
                    W["wvb"][:, jk, jd * CHW:(jd + 1) * CHW],
                    pt_sb[:, jk, :],
                    start=(jk == 0), stop=(jk == KCH - 1))
            nc.scalar.copy(st.p2t[:, h, jd, :], p2t_ps[:])


def b_parts(tc, P, W, st):
    """Phase B as three emission closures, woven into the next macro."""
    return [
        lambda: emit_b_scores(tc, P, W, st, (0, 1)),
        lambda: (emit_b_scores(tc, P, W, st, (2, 3)),
                 emit_b_softmax(tc, P, W, st)),
        lambda: emit_b_p2(tc, P, W, st),
    ]


def emit_ctx(tc, P, W, st, nch):
    """Context matmuls for 512 output rows; softmax 1/denominator folded
    into the ACT evacuation scale."""
    nc = tc.nc
    ncols = slice(nch * 512, (nch + 1) * 512)
    ctx = P.pC.tile([DQ, H, 512], BF16, tag="ctx", name="ctx", bufs=CTX_BUFS)
    for h in range(H):
        cx_ps = P.ps([DQ, 512], name="cx_ps")
        for jd in range(2):
            nc.tensor.matmul(cx_ps[:], st.p2t[:, h, jd, :],
                             st.eaT[:, 2 * h + jd, ncols],
                             start=(jd == 0), stop=(jd == 1))
        nc.scalar.activation(ctx[:, h, :], cx_ps[:], AF.Copy,
                             scale=st.rec[:, h:h + 1])
    return ctx


def emit_oproj(tc, P, W, aps, nch, ctx):
    """Output projection for 512 rows: accumulate over heads, store."""
    nc = tc.nc
    for t in range(4):
        i = nch * 4 + t
        rows = slice(i * PT, (i + 1) * PT)
        o_ps = P.ps([PT, C], name="o_ps")
        for h in range(H):
            nc.tensor.matmul(o_ps[:], ctx[:, h, t * PT:(t + 1) * PT],
                             W["wotb"][:, h, :],
                             start=(h == 0), stop=(h == H - 1))
        o_sb = P.pC.tile([PT, C], F32, tag="o_sb", name="o_sb", bufs=OSB_BUFS)
        nc.vector.tensor_copy(o_sb[:], o_ps[:])
        nc.sync.dma_start(aps["o1"][rows, :], o_sb[:])


def emit_macro(tc, P, W, aps, cur, prev):
    """One steady-state macro step: phase A of `cur` interleaved with
    phase C of `prev`, then phase B of `cur`."""
    pend = []  # (nch, ctx) emitted but not yet projected
    for ii in range(NG):
        emit_a_group(tc, P, W, aps, cur, ii)
        if prev is not None:
            pend.append((ii, emit_ctx(tc, P, W, prev, ii)))
            if len(pend) > 1:
                nch, ctx = pend.pop(0)
                emit_oproj(tc, P, W, aps, nch, ctx)
    if prev is not None:
        for nch, ctx in pend:
            emit_oproj(tc, P, W, aps, nch, ctx)
    emit_b(tc, P, W, aps, cur)


def emit_tail(tc, P, W, aps, last):
    """Drain the pipeline: phase C of the final rep."""
    pend = []
    for ii in range(NG):
        pend.append((ii, emit_ctx(tc, P, W, last, ii)))
        if len(pend) > 1:
            nch, ctx = pend.pop(0)
            emit_oproj(tc, P, W, aps, nch, ctx)
    for nch, ctx in pend:
        emit_oproj(tc, P, W, aps, nch, ctx)


_NC_CACHE = None


def get_nc():
    global _NC_CACHE
    if _NC_CACHE is None:
        _NC_CACHE = build_nc()
    return _NC_CACHE


def make_in_maps(emb1, emb_all, Wq, Wk, Wv, Wo):
    emb1 = np.ascontiguousarray(np.asarray(emb1, dtype=np.float32))
    emb_all = np.ascontiguousarray(np.asarray(emb_all, dtype=np.float32))
    Wq = np.asarray(Wq, dtype=np.float32)
    Wk = np.asarray(Wk, dtype=np.float32)
    Wv = np.asarray(Wv, dtype=np.float32)
    Wo = np.asarray(Wo, dtype=np.float32)

    scale = 1.0 / np.sqrt(np.float32(KV))
    wqt_np = np.ascontiguousarray(np.transpose(Wq, (2, 0, 1)) * scale)  # [c,h,e]
    wkt_np = np.ascontiguousarray(Wk.T)                                 # [k,ek]
    wvb_np = np.ascontiguousarray(Wv).astype(ml_dtypes.bfloat16)        # [k,d]
    wotb_np = np.ascontiguousarray(
        Wo.reshape(C, DQ, H).transpose(1, 2, 0)).astype(ml_dtypes.bfloat16)
    idb_np = np.eye(PT, dtype=ml_dtypes.bfloat16)

    shared = {"wqt": wqt_np, "wkt": wkt_np, "wvb": wvb_np, "wotb": wotb_np,
              "idb": idb_np}
    return [
        {"e1": emb1[b], "ea": emb_all[b], **shared}
        for b in range(B)
    ]


def run(inputs, trace=False, **spmd_kwargs):
    nc = get_nc()
    in_maps = make_in_maps(**inputs)
    res = run_bass_kernel_spmd(nc, in_maps, list(range(B)), trace=trace,
                               **spmd_kwargs)
    out = np.stack([np.asarray(res.results[b]["o1"]) for b in range(B)], axis=0)
    return out.astype(np.float32, copy=False), res


def kernel(**inputs) -> np.ndarray:
    out, _ = run(inputs, trace=False)
    return out
